# revision 3
# baseline (speedup 1.0000x reference)
"""Trainium2 Bass kernel for nn_Centroids (segment-mean + EMA update), v2.

Math (matches the jax reference):
    m       = y_mask
    sums[c] = sum_{i: y_i==c, m_i} x_i
    cnt[c]  = sum_{i: y_i==c} m_i
    present = any(y_i == c)  (regardless of mask)
    avg     = sums / max(cnt, 1)
    out     = where(present, DECAY*avg + (1-DECAY)*centroids, centroids)

Device algorithm (class-sharded, sorted layout):
    The host sorts rows by class, zeroes masked rows, pads every class
    segment to a multiple of B=64 columns, and ships x TRANSPOSED as
    [128 features, PC cols] fp16 per core (each core owns a contiguous
    range of <=128 classes; all rows of a class land on one core, so no
    collective is needed).  On device:
      stage A: DVE grouped reduce  [128, g, B] -> [128, g]  block sums
      stage B: PE transpose of each 128-block group + a tiny one-hot
               matmul over blocks (oh[j, c] = (blk_class[j] == c)) that
               scatter-adds block sums into per-class PSUM, with the
               per-block row counts riding along as a 2-wide matmul.
      epilogue: avg = sums * (1/max(cnt,1)); EMA blend; DMA out.
    Block/class metadata (per-block class id + per-block row counts) are
    fixed-shape f16 side inputs computed by the host while laying out
    the sorted array.
"""

import sys

for _p in ("/opt/trn_rl_repo",):
    if _p not in sys.path:
        sys.path.insert(0, _p)

from contextlib import ExitStack

import numpy as np

import concourse.bass as bass
import concourse.bacc as bacc
import concourse.mybir as mybir
import concourse.tile as tile
from concourse.bass_utils import run_bass_kernel_spmd

f32 = mybir.dt.float32
f16 = mybir.dt.float16
Alu = mybir.AluOpType
Act = mybir.ActivationFunctionType
Ax = mybir.AxisListType

# Problem constants (hardcoded per harness contract)
N = 2_000_000
D = 128
C = 1000
DECAY = 0.3
NCORES = 8

B = 64            # block size (columns); every class padded to a multiple
CH = 16384        # chunk columns per DMA (multiple of TILE_BLK)
TILE_BLK = 4096   # one j-tile = 128 S-entries = 64 blocks = 4096 columns
SENT_CLS = 200.0  # block-class sentinel for padding blocks (outside 0..127)


def chunk_plan(pc):
    """Chunk sizes: 8192 head/tail (short pipeline fill + drain), 16384 mid,
    plus a final sub-8192 remainder chunk (multiple of 64) if pc needs it."""
    assert pc % B == 0
    rem = pc % TILE_BLK
    n4 = pc // TILE_BLK
    head = min(2, max(0, n4 - 8))
    tail = min(2, max(0, n4 - head - 4))
    mid = n4 - head - tail
    plan = [TILE_BLK] * head + [4 * TILE_BLK] * (mid // 4)
    for _ in range(mid % 4):
        plan.append(TILE_BLK)
    plan += [TILE_BLK] * tail
    if rem:
        plan.append(rem)
    return plan


def build_program(pc):
    """Build the SPMD Bass program. pc = padded columns per core (mult of 64)."""
    assert pc % B == 0
    T = -(-pc // TILE_BLK)      # number of j-tiles (last may be partial)
    chunks = []
    c0 = 0
    for cc in chunk_plan(pc):
        chunks.append((c0, cc))
        c0 += cc
    assert c0 == pc

    nc = bacc.Bacc(num_devices=NCORES)

    xT_d = nc.dram_tensor("xT", [D, pc], f16, kind="ExternalInput")
    blkcn_d = nc.dram_tensor("blkcn", [128, T], f32, kind="ExternalInput")
    cnt2_d = nc.dram_tensor("cnt2", [128, 2 * T], f16, kind="ExternalInput")
    cent_d = nc.dram_tensor("cent", [128, D], f32, kind="ExternalInput")
    out_d = nc.dram_tensor("out_local", [128, D], f32, kind="ExternalOutput")

    iota_np = np.broadcast_to(np.arange(128, dtype=np.float16), (128, 128))
    iota_d = nc.inline_tensor(np.ascontiguousarray(iota_np), name="iota_const")
    ident_d = nc.inline_tensor(np.eye(128, dtype=np.float16), name="ident_const")

    with tile.TileContext(nc) as tc, ExitStack() as ctx:
        consts = ctx.enter_context(tc.tile_pool(name="consts", bufs=1))
        lab = ctx.enter_context(tc.tile_pool(name="lab", bufs=1))
        xin = ctx.enter_context(tc.tile_pool(name="xin", bufs=4))
        tre = ctx.enter_context(tc.tile_pool(name="tre", bufs=2))
        s16p = ctx.enter_context(tc.tile_pool(name="s16", bufs=2))
        stp = ctx.enter_context(tc.tile_pool(name="stp", bufs=3))
        ohp = ctx.enter_context(tc.tile_pool(name="ohp", bufs=3))
        d2p = ctx.enter_context(tc.tile_pool(name="d2p", bufs=3))
        tpp = ctx.enter_context(tc.tile_pool(name="tpp", bufs=2, space="PSUM"))
        acc = ctx.enter_context(tc.tile_pool(name="acc", bufs=1, space="PSUM"))
        ep = ctx.enter_context(tc.tile_pool(name="ep", bufs=1))

        iota_sb = consts.tile([128, 128], f16)
        nc.sync.dma_start(iota_sb[:], iota_d.ap())
        ident_sb = consts.tile([128, 128], f16)
        nc.sync.dma_start(ident_sb[:], ident_d.ap())

        blkcn_sb = lab.tile([128, T], f32)
        nc.sync.dma_start(blkcn_sb[:], blkcn_d.ap())
        cnt2_sb = lab.tile([128, 2 * T], f16)
        nc.sync.dma_start(cnt2_sb[:], cnt2_d.ap())
        cent_sb = lab.tile([128, D], f32)
        nc.sync.dma_start(cent_sb[:], cent_d.ap())

        sums_ps = acc.tile([128, D], f32)
        cnt_ps = acc.tile([128, 2], f32)

        for ci, (c0, cc) in enumerate(chunks):
            g = cc // B                      # blocks in this chunk (mult of 128)
            xt = xin.tile([128, CH], f16, tag="xt")
            nc.sync.dma_start(xt[:, :cc], xT_d.ap()[:, c0:c0 + cc])
            # pairwise-add tree (f16, 2x_1p DVE mode): [128, g, 64] -> [128, g, 2]
            # (stops at width 2: the pair shares its class in stage B, so the
            # one-hot matmul's PSUM accumulation performs the final fold free)
            ne = 2 * g                       # S-entries this chunk
            s16 = s16p.tile([128, 2 * (CH // B)], f16, tag="s16")
            cur = xt[:, :cc].rearrange("p (g b) -> p g b", b=B)
            w = B
            while w > 4:
                h = w // 2
                nt = tre.tile([128, (CH // B) * h], f16, tag=f"h{h}")
                nh = nt[:, : g * h].rearrange("p (g b) -> p g b", b=h)
                nc.vector.tensor_tensor(nh, cur[:, :, :h], cur[:, :, h:], Alu.add)
                cur = nh
                w = h
            nc.vector.tensor_tensor(
                s16[:, :ne].rearrange("p (g b) -> p g b", b=2),
                cur[:, :, 0:2],
                cur[:, :, 2:4],
                Alu.add,
            )
            n_t = -(-ne // 128)
            if ne % 128:
                # zero the s16 tail so the partial final j-tile transposes to
                # all-zero rows for the sentinel entries
                nc.vector.memset(s16[:, ne:n_t * 128], 0)
            for k in range(n_t):
                tt = (c0 // TILE_BLK) + k
                tp = tpp.tile([128, 128], f16, tag="tp")
                nc.tensor.transpose(tp[:], s16[:, k * 128:(k + 1) * 128], ident_sb[:])
                st = stp.tile([128, 128], f16, tag="st")
                nc.scalar.copy(st[:], tp[:])
                # one-hot on the ACT engine: oh = Relu(1 - |iota - class|)
                d2 = d2p.tile([128, 128], f16, tag="d2")
                nc.scalar.activation(
                    d2[:], iota_sb[:], Act.Abs, bias=blkcn_sb[:, tt:tt + 1]
                )
                oh = ohp.tile([128, 128], f16, tag="oh")
                nc.scalar.activation(oh[:], d2[:], Act.Relu, bias=1.0, scale=-1.0)
                first = tt == 0
                last = tt == T - 1
                nc.tensor.matmul(sums_ps[:], oh[:], st[:], start=first, stop=last)
                nc.tensor.matmul(
                    cnt_ps[:], oh[:], cnt2_sb[:, 2 * tt:2 * tt + 2],
                    start=first, stop=last,
                )

        # ---- EMA epilogue ----
        cnt_sb = ep.tile([128, 2], f32)
        nc.vector.tensor_copy(cnt_sb[:], cnt_ps[:])
        den = ep.tile([128, 1], f32)
        nc.vector.tensor_scalar_max(den[:], cnt_sb[:, 0:1], 1.0)
        rec = ep.tile([128, 1], f32)
        nc.vector.reciprocal(rec[:], den[:])
        pres = ep.tile([128, 1], f32)
        nc.vector.tensor_scalar(pres[:], cnt_sb[:, 1:2], 0.5, DECAY, Alu.is_gt, Alu.mult)
        avg = ep.tile([128, D], f32)
        nc.vector.tensor_scalar_mul(avg[:], sums_ps[:], rec[:])
        dlt = ep.tile([128, D], f32)
        nc.vector.tensor_sub(dlt[:], avg[:], cent_sb[:])
        sc2 = ep.tile([128, D], f32)
        nc.vector.tensor_scalar_mul(sc2[:], dlt[:], pres[:])
        oc = ep.tile([128, D], f32)
        nc.vector.tensor_add(oc[:], sc2[:], cent_sb[:])
        nc.sync.dma_start(out_d.ap()[:, :], oc[:])

    nc.compile()
    return nc


_NC_CACHE = {}


def get_program(pc):
    if pc not in _NC_CACHE:
        _NC_CACHE[pc] = build_program(pc)
    return _NC_CACHE[pc]


def _roundup(v, q):
    return (v + q - 1) // q * q


def make_in_maps(x, y, y_mask, centroids):
    y = np.asarray(y).astype(np.int64, copy=False).ravel()
    m = np.asarray(y_mask).astype(bool, copy=False).ravel()
    x16 = np.asarray(x, dtype=np.float32).astype(np.float16)
    if not m.all():
        x16 = x16.copy()
        x16[~m] = 0
    cent = np.asarray(centroids, dtype=np.float32)

    order = np.argsort(y, kind="stable")
    ys = y[order]
    ms = m[order]

    cnt_raw = np.bincount(y, minlength=C).astype(np.int64)
    pad_cnt = ((cnt_raw + B - 1) // B) * B
    csum = np.concatenate([[0], np.cumsum(pad_cnt)])
    total = csum[-1]
    # contiguous class ranges, ~balanced by padded column count, <=128 classes
    bounds = [0]
    for k in range(1, NCORES):
        t = total * k // NCORES
        b = int(np.searchsorted(csum, t))
        b = max(bounds[-1] + 1, min(b, C - (NCORES - k)))
        b = min(b, bounds[-1] + 128)
        bounds.append(b)
    bounds.append(C)
    for k in range(NCORES):
        assert bounds[k + 1] - bounds[k] <= 128

    cols_per_core = [int(csum[bounds[k + 1]] - csum[bounds[k]]) for k in range(NCORES)]
    pc = _roundup(max(cols_per_core), B)
    T = -(-pc // TILE_BLK)
    E = T * 128          # padded S-entries (2 per 64-col block + sentinel tail)

    class_start = np.concatenate([[0], np.cumsum(cnt_raw)])  # in sorted order

    in_maps = []
    meta = []
    for k in range(NCORES):
        ca, cb = bounds[k], bounds[k + 1]
        ncls = cb - ca
        pos_a, pos_b = int(class_start[ca]), int(class_start[cb])
        ysk = ys[pos_a:pos_b]
        # column index for each sorted row of this core
        off_local = (csum[ca:cb] - csum[ca]).astype(np.int64)
        rank = np.arange(pos_a, pos_b, dtype=np.int64) - class_start[ysk]
        cols = off_local[ysk - ca] + rank

        padded = np.zeros((pc, D), dtype=np.float16)
        padded[cols] = x16[order[pos_a:pos_b]]
        xT = np.ascontiguousarray(padded.T)

        # per-block metadata
        nb = (pad_cnt[ca:cb] // B).astype(np.int64)
        bcb = np.full(E // 2, SENT_CLS, dtype=np.float32)   # per 64-col block
        lab_rep = np.repeat(np.arange(ncls, dtype=np.float32), nb)
        bcb[: lab_rep.size] = lab_rep
        rawcol = np.zeros((E // 2) * B, dtype=np.float32)
        rawcol[cols] = 1.0
        mskcol = np.zeros((E // 2) * B, dtype=np.float32)
        mskcol[cols] = ms[pos_a:pos_b].astype(np.float32)
        raw_b = rawcol.reshape(E // 2, B).sum(1)
        msk_b = mskcol.reshape(E // 2, B).sum(1)

        # per-entry metadata: both entries of a block share its class; the
        # block's row counts ride on the even entry only
        ent_class = np.repeat(bcb, 2)
        ent_cm = np.zeros(E, dtype=np.float32)
        ent_cm[0::2] = msk_b
        ent_cr = np.zeros(E, dtype=np.float32)
        ent_cr[0::2] = raw_b

        blkcn = np.ascontiguousarray((-ent_class).reshape(T, 128).T).astype(np.float32)
        cnt2 = np.empty((128, 2 * T), dtype=np.float16)
        cnt2[:, 0::2] = ent_cm.reshape(T, 128).T
        cnt2[:, 1::2] = ent_cr.reshape(T, 128).T

        cent_local = np.zeros((128, D), dtype=np.float32)
        cent_local[:ncls] = cent[ca:cb]

        in_maps.append(
            {
                "xT": xT,
                "blkcn": blkcn,
                "cnt2": cnt2,
                "cent": cent_local,
            }
        )
        meta.append((ca, cb))
    return in_maps, meta, pc


def run(x, y, y_mask, centroids, **spmd_kwargs):
    in_maps, meta, pc = make_in_maps(x, y, y_mask, centroids)
    nc = get_program(pc)
    res = run_bass_kernel_spmd(nc, in_maps, list(range(NCORES)), **spmd_kwargs)
    out = np.array(np.asarray(centroids, dtype=np.float32), copy=True)
    for k, (ca, cb) in enumerate(meta):
        out[ca:cb] = res.results[k]["out_local"][: cb - ca]
    return out, res


def kernel(x, y, y_mask, centroids):
    out, _ = run(x, y, y_mask, centroids)
    return out


# revision 4
# speedup vs baseline: 1.0132x; 1.0132x over previous
"""Trainium2 Bass kernel for nn_Centroids (segment-mean + EMA update), v2.

Math (matches the jax reference):
    m       = y_mask
    sums[c] = sum_{i: y_i==c, m_i} x_i
    cnt[c]  = sum_{i: y_i==c} m_i
    present = any(y_i == c)  (regardless of mask)
    avg     = sums / max(cnt, 1)
    out     = where(present, DECAY*avg + (1-DECAY)*centroids, centroids)

Device algorithm (class-sharded, sorted layout):
    The host sorts rows by class, zeroes masked rows, pads every class
    segment to a multiple of B=64 columns, and ships x TRANSPOSED as
    [128 features, PC cols] fp16 per core (each core owns a contiguous
    range of <=128 classes; all rows of a class land on one core, so no
    collective is needed).  On device, per double-buffered column chunk:
      stage A: DVE pairwise-add tree (f16, 2x_1p fast mode; TensorReduce
               has no fast mode) folds [128, g, 64] down to width 2 —
               two S-entries per block; the final fold happens for free
               in stage B's PSUM accumulation since both entries of a
               block carry the same class.
      stage B: PE transpose of each 128-entry group, ACT-engine one-hot
               (Relu(1 - |iota - class|)), then a tiny matmul
               scatter-adds entries into per-class PSUM, with per-entry
               row counts riding along as a 2-wide matmul.
      epilogue: avg = sums * (1/max(cnt,1)); EMA blend; DMA out.
    Entry/class metadata (class id + row counts per S-entry) are
    fixed-shape side inputs computed by the host while laying out the
    sorted array.  All DMAs stay on the sync queue (ACT/gpsimd-issued
    DMAs measured slower); xin bufs=4 keeps the 16 HWDGE queues fed
    (bufs=3 was bistable: ~182us fed vs ~215us starved).
"""

import sys

for _p in ("/opt/trn_rl_repo",):
    if _p not in sys.path:
        sys.path.insert(0, _p)

from contextlib import ExitStack

import numpy as np

import concourse.bass as bass
import concourse.bacc as bacc
import concourse.mybir as mybir
import concourse.tile as tile
from concourse.bass_utils import run_bass_kernel_spmd

f32 = mybir.dt.float32
f16 = mybir.dt.float16
Alu = mybir.AluOpType
Act = mybir.ActivationFunctionType
Ax = mybir.AxisListType

# Problem constants (hardcoded per harness contract)
N = 2_000_000
D = 128
C = 1000
DECAY = 0.3
NCORES = 8

B = 64            # block size (columns); every class padded to a multiple
CH = 16384        # chunk columns per DMA (multiple of TILE_BLK)
TILE_BLK = 4096   # one j-tile = 128 S-entries = 64 blocks = 4096 columns
SENT_CLS = 200.0  # block-class sentinel for padding blocks (outside 0..127)


def chunk_plan(pc):
    """Chunk sizes: 8192 head/tail (short pipeline fill + drain), 16384 mid,
    plus a final sub-8192 remainder chunk (multiple of 64) if pc needs it."""
    assert pc % B == 0
    rem = pc % TILE_BLK
    n4 = pc // TILE_BLK
    head = min(2, max(0, n4 - 8))
    tail = min(2, max(0, n4 - head - 4))
    mid = n4 - head - tail
    plan = [TILE_BLK] * head + [4 * TILE_BLK] * (mid // 4)
    for _ in range(mid % 4):
        plan.append(TILE_BLK)
    plan += [TILE_BLK] * tail
    if rem:
        plan.append(rem)
    return plan


def build_program(pc):
    """Build the SPMD Bass program. pc = padded columns per core (mult of 64)."""
    assert pc % B == 0
    T = -(-pc // TILE_BLK)      # number of j-tiles (last may be partial)
    chunks = []
    c0 = 0
    for cc in chunk_plan(pc):
        chunks.append((c0, cc))
        c0 += cc
    assert c0 == pc

    nc = bacc.Bacc(num_devices=NCORES)

    xT_d = nc.dram_tensor("xT", [D, pc], f16, kind="ExternalInput")
    blkcn_d = nc.dram_tensor("blkcn", [128, T], f32, kind="ExternalInput")
    cnt2_d = nc.dram_tensor("cnt2", [128, 2 * T], f16, kind="ExternalInput")
    cent_d = nc.dram_tensor("cent", [128, D], f32, kind="ExternalInput")
    out_d = nc.dram_tensor("out_local", [128, D], f32, kind="ExternalOutput")

    iota_np = np.broadcast_to(np.arange(128, dtype=np.float16), (128, 128))
    iota_d = nc.inline_tensor(np.ascontiguousarray(iota_np), name="iota_const")
    ident_d = nc.inline_tensor(np.eye(128, dtype=np.float16), name="ident_const")

    with tile.TileContext(nc) as tc, ExitStack() as ctx:
        consts = ctx.enter_context(tc.tile_pool(name="consts", bufs=1))
        lab = ctx.enter_context(tc.tile_pool(name="lab", bufs=1))
        xin = ctx.enter_context(tc.tile_pool(name="xin", bufs=4))
        tre = ctx.enter_context(tc.tile_pool(name="tre", bufs=2))
        s16p = ctx.enter_context(tc.tile_pool(name="s16", bufs=2))
        stp = ctx.enter_context(tc.tile_pool(name="stp", bufs=3))
        ohp = ctx.enter_context(tc.tile_pool(name="ohp", bufs=3))
        d2p = ctx.enter_context(tc.tile_pool(name="d2p", bufs=3))
        tpp = ctx.enter_context(tc.tile_pool(name="tpp", bufs=2, space="PSUM"))
        acc = ctx.enter_context(tc.tile_pool(name="acc", bufs=1, space="PSUM"))
        ep = ctx.enter_context(tc.tile_pool(name="ep", bufs=1))

        iota_sb = consts.tile([128, 128], f16)
        nc.sync.dma_start(iota_sb[:], iota_d.ap())
        ident_sb = consts.tile([128, 128], f16)
        nc.sync.dma_start(ident_sb[:], ident_d.ap())

        blkcn_sb = lab.tile([128, T], f32)
        nc.sync.dma_start(blkcn_sb[:], blkcn_d.ap())
        cnt2_sb = lab.tile([128, 2 * T], f16)
        nc.sync.dma_start(cnt2_sb[:], cnt2_d.ap())
        cent_sb = lab.tile([128, D], f32)
        nc.sync.dma_start(cent_sb[:], cent_d.ap())

        sums_ps = acc.tile([128, D], f32)
        cnt_ps = acc.tile([128, 2], f32)

        for ci, (c0, cc) in enumerate(chunks):
            g = cc // B                      # blocks in this chunk (mult of 128)
            xt = xin.tile([128, CH], f16, tag="xt")
            nc.sync.dma_start(xt[:, :cc], xT_d.ap()[:, c0:c0 + cc])
            # pairwise-add tree (f16, 2x_1p DVE mode): [128, g, 64] -> [128, g, 2]
            # (stops at width 2: the pair shares its class in stage B, so the
            # one-hot matmul's PSUM accumulation performs the final fold free)
            ne = 2 * g                       # S-entries this chunk
            s16 = s16p.tile([128, 2 * (CH // B)], f16, tag="s16")
            cur = xt[:, :cc].rearrange("p (g b) -> p g b", b=B)
            w = B
            while w > 4:
                h = w // 2
                nt = tre.tile([128, (CH // B) * h], f16, tag=f"h{h}")
                nh = nt[:, : g * h].rearrange("p (g b) -> p g b", b=h)
                nc.vector.tensor_tensor(nh, cur[:, :, :h], cur[:, :, h:], Alu.add)
                cur = nh
                w = h
            nc.vector.tensor_tensor(
                s16[:, :ne].rearrange("p (g b) -> p g b", b=2),
                cur[:, :, 0:2],
                cur[:, :, 2:4],
                Alu.add,
            )
            n_t = -(-ne // 128)
            if ne % 128:
                # zero the s16 tail so the partial final j-tile transposes to
                # all-zero rows for the sentinel entries
                nc.vector.memset(s16[:, ne:n_t * 128], 0)
            for k in range(n_t):
                tt = (c0 // TILE_BLK) + k
                tp = tpp.tile([128, 128], f16, tag="tp")
                nc.tensor.transpose(tp[:], s16[:, k * 128:(k + 1) * 128], ident_sb[:])
                st = stp.tile([128, 128], f16, tag="st")
                nc.scalar.copy(st[:], tp[:])
                # one-hot on the ACT engine: oh = Relu(1 - |iota - class|)
                d2 = d2p.tile([128, 128], f16, tag="d2")
                nc.scalar.activation(
                    d2[:], iota_sb[:], Act.Abs, bias=blkcn_sb[:, tt:tt + 1]
                )
                oh = ohp.tile([128, 128], f16, tag="oh")
                nc.scalar.activation(oh[:], d2[:], Act.Relu, bias=1.0, scale=-1.0)
                first = tt == 0
                last = tt == T - 1
                nc.tensor.matmul(sums_ps[:], oh[:], st[:], start=first, stop=last)
                nc.tensor.matmul(
                    cnt_ps[:], oh[:], cnt2_sb[:, 2 * tt:2 * tt + 2],
                    start=first, stop=last,
                )

        # ---- EMA epilogue ----
        cnt_sb = ep.tile([128, 2], f32)
        nc.vector.tensor_copy(cnt_sb[:], cnt_ps[:])
        den = ep.tile([128, 1], f32)
        nc.vector.tensor_scalar_max(den[:], cnt_sb[:, 0:1], 1.0)
        rec = ep.tile([128, 1], f32)
        nc.vector.reciprocal(rec[:], den[:])
        pres = ep.tile([128, 1], f32)
        nc.vector.tensor_scalar(pres[:], cnt_sb[:, 1:2], 0.5, DECAY, Alu.is_gt, Alu.mult)
        avg = ep.tile([128, D], f32)
        nc.vector.tensor_scalar_mul(avg[:], sums_ps[:], rec[:])
        dlt = ep.tile([128, D], f32)
        nc.vector.tensor_sub(dlt[:], avg[:], cent_sb[:])
        sc2 = ep.tile([128, D], f32)
        nc.vector.tensor_scalar_mul(sc2[:], dlt[:], pres[:])
        oc = ep.tile([128, D], f32)
        nc.vector.tensor_add(oc[:], sc2[:], cent_sb[:])
        nc.sync.dma_start(out_d.ap()[:, :], oc[:])

    nc.compile()
    return nc


_NC_CACHE = {}


def get_program(pc):
    if pc not in _NC_CACHE:
        _NC_CACHE[pc] = build_program(pc)
    return _NC_CACHE[pc]


def _roundup(v, q):
    return (v + q - 1) // q * q


def make_in_maps(x, y, y_mask, centroids):
    y = np.asarray(y).astype(np.int64, copy=False).ravel()
    m = np.asarray(y_mask).astype(bool, copy=False).ravel()
    x16 = np.asarray(x, dtype=np.float32).astype(np.float16)
    if not m.all():
        x16 = x16.copy()
        x16[~m] = 0
    cent = np.asarray(centroids, dtype=np.float32)

    order = np.argsort(y, kind="stable")
    ys = y[order]
    ms = m[order]

    cnt_raw = np.bincount(y, minlength=C).astype(np.int64)
    pad_cnt = ((cnt_raw + B - 1) // B) * B
    csum = np.concatenate([[0], np.cumsum(pad_cnt)])
    total = csum[-1]
    # contiguous class ranges, ~balanced by padded column count, <=128 classes
    bounds = [0]
    for k in range(1, NCORES):
        t = total * k // NCORES
        b = int(np.searchsorted(csum, t))
        b = max(bounds[-1] + 1, min(b, C - (NCORES - k)))
        b = min(b, bounds[-1] + 128)
        bounds.append(b)
    bounds.append(C)
    for k in range(NCORES):
        assert bounds[k + 1] - bounds[k] <= 128

    cols_per_core = [int(csum[bounds[k + 1]] - csum[bounds[k]]) for k in range(NCORES)]
    pc = _roundup(max(cols_per_core), B)
    T = -(-pc // TILE_BLK)
    E = T * 128          # padded S-entries (2 per 64-col block + sentinel tail)

    class_start = np.concatenate([[0], np.cumsum(cnt_raw)])  # in sorted order

    in_maps = []
    meta = []
    for k in range(NCORES):
        ca, cb = bounds[k], bounds[k + 1]
        ncls = cb - ca
        pos_a, pos_b = int(class_start[ca]), int(class_start[cb])
        ysk = ys[pos_a:pos_b]
        # column index for each sorted row of this core
        off_local = (csum[ca:cb] - csum[ca]).astype(np.int64)
        rank = np.arange(pos_a, pos_b, dtype=np.int64) - class_start[ysk]
        cols = off_local[ysk - ca] + rank

        padded = np.zeros((pc, D), dtype=np.float16)
        padded[cols] = x16[order[pos_a:pos_b]]
        xT = np.ascontiguousarray(padded.T)

        # per-block metadata
        nb = (pad_cnt[ca:cb] // B).astype(np.int64)
        bcb = np.full(E // 2, SENT_CLS, dtype=np.float32)   # per 64-col block
        lab_rep = np.repeat(np.arange(ncls, dtype=np.float32), nb)
        bcb[: lab_rep.size] = lab_rep
        rawcol = np.zeros((E // 2) * B, dtype=np.float32)
        rawcol[cols] = 1.0
        mskcol = np.zeros((E // 2) * B, dtype=np.float32)
        mskcol[cols] = ms[pos_a:pos_b].astype(np.float32)
        raw_b = rawcol.reshape(E // 2, B).sum(1)
        msk_b = mskcol.reshape(E // 2, B).sum(1)

        # per-entry metadata: both entries of a block share its class; the
        # block's row counts ride on the even entry only
        ent_class = np.repeat(bcb, 2)
        ent_cm = np.zeros(E, dtype=np.float32)
        ent_cm[0::2] = msk_b
        ent_cr = np.zeros(E, dtype=np.float32)
        ent_cr[0::2] = raw_b

        blkcn = np.ascontiguousarray((-ent_class).reshape(T, 128).T).astype(np.float32)
        cnt2 = np.empty((128, 2 * T), dtype=np.float16)
        cnt2[:, 0::2] = ent_cm.reshape(T, 128).T
        cnt2[:, 1::2] = ent_cr.reshape(T, 128).T

        cent_local = np.zeros((128, D), dtype=np.float32)
        cent_local[:ncls] = cent[ca:cb]

        in_maps.append(
            {
                "xT": xT,
                "blkcn": blkcn,
                "cnt2": cnt2,
                "cent": cent_local,
            }
        )
        meta.append((ca, cb))
    return in_maps, meta, pc


def run(x, y, y_mask, centroids, **spmd_kwargs):
    in_maps, meta, pc = make_in_maps(x, y, y_mask, centroids)
    nc = get_program(pc)
    res = run_bass_kernel_spmd(nc, in_maps, list(range(NCORES)), **spmd_kwargs)
    out = np.array(np.asarray(centroids, dtype=np.float32), copy=True)
    for k, (ca, cb) in enumerate(meta):
        out[ca:cb] = res.results[k]["out_local"][: cb - ca]
    return out, res


def kernel(x, y, y_mask, centroids):
    out, _ = run(x, y, y_mask, centroids)
    return out


# revision 5
# speedup vs baseline: 1.2018x; 1.1862x over previous
"""Trainium2 Bass kernel for nn_Centroids (segment-mean + EMA update), v2.

Math (matches the jax reference):
    m       = y_mask
    sums[c] = sum_{i: y_i==c, m_i} x_i
    cnt[c]  = sum_{i: y_i==c} m_i
    present = any(y_i == c)  (regardless of mask)
    avg     = sums / max(cnt, 1)
    out     = where(present, DECAY*avg + (1-DECAY)*centroids, centroids)

Device algorithm (class-sharded, sorted layout):
    The host sorts rows by class, zeroes masked rows, pads every class
    segment to a multiple of B=64 columns, and ships x TRANSPOSED as
    [128 features, PC cols] fp16 per core (each core owns a contiguous
    range of <=128 classes; all rows of a class land on one core, so no
    collective is needed).  On device, per double-buffered column chunk:
      stage A: DVE pairwise-add tree (f16, 2x_1p fast mode; TensorReduce
               has no fast mode) folds [128, g, 64] down to width 2 —
               two S-entries per block; the final fold happens for free
               in stage B's PSUM accumulation since both entries of a
               block carry the same class.
      stage B: PE transpose of each 128-entry group, ACT-engine one-hot
               (Relu(1 - |iota - class|)), then a tiny matmul
               scatter-adds entries into per-class PSUM, with per-entry
               row counts riding along as a 2-wide matmul.
      epilogue: avg = sums * (1/max(cnt,1)); EMA blend; DMA out.
    Entry/class metadata (class id + row counts per S-entry) are
    fixed-shape side inputs computed by the host while laying out the
    sorted array.  All DMAs stay on the sync queue (ACT-issued DMAs
    reproducibly kill the fast mode; gpsimd SWDGE is neutral); the first
    three x chunks are issued ahead of the const DMAs so the DVE starts
    sooner; xin bufs=4 keeps the 16 HWDGE queues fed (bufs=3 was
    bistable: ~182us fed vs ~215us starved).
"""

import sys

for _p in ("/opt/trn_rl_repo",):
    if _p not in sys.path:
        sys.path.insert(0, _p)

from contextlib import ExitStack

import numpy as np

import concourse.bass as bass
import concourse.bacc as bacc
import concourse.mybir as mybir
import concourse.tile as tile
from concourse.bass_utils import run_bass_kernel_spmd

f32 = mybir.dt.float32
f16 = mybir.dt.float16
Alu = mybir.AluOpType
Act = mybir.ActivationFunctionType
Ax = mybir.AxisListType

# Problem constants (hardcoded per harness contract)
N = 2_000_000
D = 128
C = 1000
DECAY = 0.3
NCORES = 8

B = 64            # block size (columns); every class padded to a multiple
CH = 16384        # chunk columns per DMA (multiple of TILE_BLK)
TILE_BLK = 4096   # one j-tile = 128 S-entries = 64 blocks = 4096 columns
SENT_CLS = 200.0  # block-class sentinel for padding blocks (outside 0..127)


def chunk_plan(pc):
    """Chunk sizes: 8192 head/tail (short pipeline fill + drain), 16384 mid,
    plus a final sub-8192 remainder chunk (multiple of 64) if pc needs it."""
    assert pc % B == 0
    rem = pc % TILE_BLK
    n4 = pc // TILE_BLK
    head = min(2, max(0, n4 - 8))
    tail = min(2, max(0, n4 - head - 4))
    mid = n4 - head - tail
    plan = [TILE_BLK] * head + [4 * TILE_BLK] * (mid // 4)
    for _ in range(mid % 4):
        plan.append(TILE_BLK)
    plan += [TILE_BLK] * tail
    if rem:
        plan.append(rem)
    return plan


def build_program(pc):
    """Build the SPMD Bass program. pc = padded columns per core (mult of 64)."""
    assert pc % B == 0
    T = -(-pc // TILE_BLK)      # number of j-tiles (last may be partial)
    chunks = []
    c0 = 0
    for cc in chunk_plan(pc):
        chunks.append((c0, cc))
        c0 += cc
    assert c0 == pc

    nc = bacc.Bacc(num_devices=NCORES)

    xT_d = nc.dram_tensor("xT", [D, pc], f16, kind="ExternalInput")
    blkcn_d = nc.dram_tensor("blkcn", [128, T], f32, kind="ExternalInput")
    cnt2_d = nc.dram_tensor("cnt2", [128, 2 * T], f16, kind="ExternalInput")
    cent_d = nc.dram_tensor("cent", [128, D], f32, kind="ExternalInput")
    out_d = nc.dram_tensor("out_local", [128, D], f32, kind="ExternalOutput")

    iota_np = np.broadcast_to(np.arange(128, dtype=np.float16), (128, 128))
    iota_d = nc.inline_tensor(np.ascontiguousarray(iota_np), name="iota_const")
    ident_d = nc.inline_tensor(np.eye(128, dtype=np.float16), name="ident_const")

    with tile.TileContext(nc) as tc, ExitStack() as ctx:
        consts = ctx.enter_context(tc.tile_pool(name="consts", bufs=1))
        lab = ctx.enter_context(tc.tile_pool(name="lab", bufs=1))
        xin = ctx.enter_context(tc.tile_pool(name="xin", bufs=4))
        tre = ctx.enter_context(tc.tile_pool(name="tre", bufs=2))
        s16p = ctx.enter_context(tc.tile_pool(name="s16", bufs=2))
        stp = ctx.enter_context(tc.tile_pool(name="stp", bufs=3))
        ohp = ctx.enter_context(tc.tile_pool(name="ohp", bufs=3))
        d2p = ctx.enter_context(tc.tile_pool(name="d2p", bufs=3))
        tpp = ctx.enter_context(tc.tile_pool(name="tpp", bufs=2, space="PSUM"))
        acc = ctx.enter_context(tc.tile_pool(name="acc", bufs=1, space="PSUM"))
        ep = ctx.enter_context(tc.tile_pool(name="ep", bufs=1))

        # lead the DMA queues with the first x chunks; consts follow (they
        # are not needed until stage B of chunk 0)
        pre_xts = {}
        for ci in range(min(3, len(chunks))):
            c0, cc = chunks[ci]
            xt = xin.tile([128, CH], f16, tag="xt")
            nc.sync.dma_start(xt[:, :cc], xT_d.ap()[:, c0:c0 + cc])
            pre_xts[ci] = xt

        iota_sb = consts.tile([128, 128], f16)
        nc.sync.dma_start(iota_sb[:], iota_d.ap())
        ident_sb = consts.tile([128, 128], f16)
        nc.sync.dma_start(ident_sb[:], ident_d.ap())

        blkcn_sb = lab.tile([128, T], f32)
        nc.sync.dma_start(blkcn_sb[:], blkcn_d.ap())
        cnt2_sb = lab.tile([128, 2 * T], f16)
        nc.sync.dma_start(cnt2_sb[:], cnt2_d.ap())
        cent_sb = lab.tile([128, D], f32)
        nc.sync.dma_start(cent_sb[:], cent_d.ap())

        sums_ps = acc.tile([128, D], f32)
        cnt_ps = acc.tile([128, 2], f32)

        for ci, (c0, cc) in enumerate(chunks):
            g = cc // B
            if ci in pre_xts:
                xt = pre_xts.pop(ci)
            else:
                xt = xin.tile([128, CH], f16, tag="xt")
                nc.sync.dma_start(xt[:, :cc], xT_d.ap()[:, c0:c0 + cc])
            # pairwise-add tree (f16, 2x_1p DVE mode): [128, g, 64] -> [128, g, 2]
            # (stops at width 2: the pair shares its class in stage B, so the
            # one-hot matmul's PSUM accumulation performs the final fold free)
            ne = 2 * g                       # S-entries this chunk
            s16 = s16p.tile([128, 2 * (CH // B)], f16, tag="s16")
            cur = xt[:, :cc].rearrange("p (g b) -> p g b", b=B)
            w = B
            while w > 4:
                h = w // 2
                nt = tre.tile([128, (CH // B) * h], f16, tag=f"h{h}")
                nh = nt[:, : g * h].rearrange("p (g b) -> p g b", b=h)
                nc.vector.tensor_tensor(nh, cur[:, :, :h], cur[:, :, h:], Alu.add)
                cur = nh
                w = h
            nc.vector.tensor_tensor(
                s16[:, :ne].rearrange("p (g b) -> p g b", b=2),
                cur[:, :, 0:2],
                cur[:, :, 2:4],
                Alu.add,
            )
            n_t = -(-ne // 128)
            if ne % 128:
                # zero the s16 tail so the partial final j-tile transposes to
                # all-zero rows for the sentinel entries
                nc.vector.memset(s16[:, ne:n_t * 128], 0)
            for k in range(n_t):
                tt = (c0 // TILE_BLK) + k
                tp = tpp.tile([128, 128], f16, tag="tp")
                nc.tensor.transpose(tp[:], s16[:, k * 128:(k + 1) * 128], ident_sb[:])
                st = stp.tile([128, 128], f16, tag="st")
                nc.scalar.copy(st[:], tp[:])
                # one-hot on the ACT engine: oh = Relu(1 - |iota - class|)
                d2 = d2p.tile([128, 128], f16, tag="d2")
                nc.scalar.activation(
                    d2[:], iota_sb[:], Act.Abs, bias=blkcn_sb[:, tt:tt + 1]
                )
                oh = ohp.tile([128, 128], f16, tag="oh")
                nc.scalar.activation(oh[:], d2[:], Act.Relu, bias=1.0, scale=-1.0)
                first = tt == 0
                last = tt == T - 1
                nc.tensor.matmul(sums_ps[:], oh[:], st[:], start=first, stop=last)
                nc.tensor.matmul(
                    cnt_ps[:], oh[:], cnt2_sb[:, 2 * tt:2 * tt + 2],
                    start=first, stop=last,
                )

        # ---- EMA epilogue ----
        cnt_sb = ep.tile([128, 2], f32)
        nc.vector.tensor_copy(cnt_sb[:], cnt_ps[:])
        den = ep.tile([128, 1], f32)
        nc.vector.tensor_scalar_max(den[:], cnt_sb[:, 0:1], 1.0)
        rec = ep.tile([128, 1], f32)
        nc.vector.reciprocal(rec[:], den[:])
        pres = ep.tile([128, 1], f32)
        nc.vector.tensor_scalar(pres[:], cnt_sb[:, 1:2], 0.5, DECAY, Alu.is_gt, Alu.mult)
        pres1 = ep.tile([128, 1], f32)
        nc.vector.tensor_scalar(pres1[:], pres[:], -1.0, 1.0, Alu.mult, Alu.add)
        # oc = cent*(1-pres) + (sums*rec)*pres, fused into two big ops
        avgp = ep.tile([128, D], f32)
        nc.vector.tensor_scalar(avgp[:], sums_ps[:], rec[:], pres[:], Alu.mult, Alu.mult)
        oc = ep.tile([128, D], f32)
        nc.vector.scalar_tensor_tensor(
            oc[:], cent_sb[:], pres1[:], avgp[:], Alu.mult, Alu.add
        )
        nc.sync.dma_start(out_d.ap()[:, :], oc[:])

    nc.compile()
    return nc


_NC_CACHE = {}


def get_program(pc):
    if pc not in _NC_CACHE:
        _NC_CACHE[pc] = build_program(pc)
    return _NC_CACHE[pc]


def _roundup(v, q):
    return (v + q - 1) // q * q


def make_in_maps(x, y, y_mask, centroids):
    y = np.asarray(y).astype(np.int64, copy=False).ravel()
    m = np.asarray(y_mask).astype(bool, copy=False).ravel()
    x16 = np.asarray(x, dtype=np.float32).astype(np.float16)
    if not m.all():
        x16 = x16.copy()
        x16[~m] = 0
    cent = np.asarray(centroids, dtype=np.float32)

    order = np.argsort(y, kind="stable")
    ys = y[order]
    ms = m[order]

    cnt_raw = np.bincount(y, minlength=C).astype(np.int64)
    pad_cnt = ((cnt_raw + B - 1) // B) * B
    csum = np.concatenate([[0], np.cumsum(pad_cnt)])
    total = csum[-1]
    # contiguous class ranges, ~balanced by padded column count, <=128 classes
    bounds = [0]
    for k in range(1, NCORES):
        t = total * k // NCORES
        b = int(np.searchsorted(csum, t))
        b = max(bounds[-1] + 1, min(b, C - (NCORES - k)))
        b = min(b, bounds[-1] + 128)
        bounds.append(b)
    bounds.append(C)
    for k in range(NCORES):
        assert bounds[k + 1] - bounds[k] <= 128

    cols_per_core = [int(csum[bounds[k + 1]] - csum[bounds[k]]) for k in range(NCORES)]
    pc = _roundup(max(cols_per_core), B)
    T = -(-pc // TILE_BLK)
    E = T * 128          # padded S-entries (2 per 64-col block + sentinel tail)

    class_start = np.concatenate([[0], np.cumsum(cnt_raw)])  # in sorted order

    in_maps = []
    meta = []
    for k in range(NCORES):
        ca, cb = bounds[k], bounds[k + 1]
        ncls = cb - ca
        pos_a, pos_b = int(class_start[ca]), int(class_start[cb])
        ysk = ys[pos_a:pos_b]
        # column index for each sorted row of this core
        off_local = (csum[ca:cb] - csum[ca]).astype(np.int64)
        rank = np.arange(pos_a, pos_b, dtype=np.int64) - class_start[ysk]
        cols = off_local[ysk - ca] + rank

        padded = np.zeros((pc, D), dtype=np.float16)
        padded[cols] = x16[order[pos_a:pos_b]]
        xT = np.ascontiguousarray(padded.T)

        # per-block metadata
        nb = (pad_cnt[ca:cb] // B).astype(np.int64)
        bcb = np.full(E // 2, SENT_CLS, dtype=np.float32)   # per 64-col block
        lab_rep = np.repeat(np.arange(ncls, dtype=np.float32), nb)
        bcb[: lab_rep.size] = lab_rep
        rawcol = np.zeros((E // 2) * B, dtype=np.float32)
        rawcol[cols] = 1.0
        mskcol = np.zeros((E // 2) * B, dtype=np.float32)
        mskcol[cols] = ms[pos_a:pos_b].astype(np.float32)
        raw_b = rawcol.reshape(E // 2, B).sum(1)
        msk_b = mskcol.reshape(E // 2, B).sum(1)

        # per-entry metadata: both entries of a block share its class; the
        # block's row counts ride on the even entry only
        ent_class = np.repeat(bcb, 2)
        ent_cm = np.zeros(E, dtype=np.float32)
        ent_cm[0::2] = msk_b
        ent_cr = np.zeros(E, dtype=np.float32)
        ent_cr[0::2] = raw_b

        blkcn = np.ascontiguousarray((-ent_class).reshape(T, 128).T).astype(np.float32)
        cnt2 = np.empty((128, 2 * T), dtype=np.float16)
        cnt2[:, 0::2] = ent_cm.reshape(T, 128).T
        cnt2[:, 1::2] = ent_cr.reshape(T, 128).T

        cent_local = np.zeros((128, D), dtype=np.float32)
        cent_local[:ncls] = cent[ca:cb]

        in_maps.append(
            {
                "xT": xT,
                "blkcn": blkcn,
                "cnt2": cnt2,
                "cent": cent_local,
            }
        )
        meta.append((ca, cb))
    return in_maps, meta, pc


def run(x, y, y_mask, centroids, **spmd_kwargs):
    in_maps, meta, pc = make_in_maps(x, y, y_mask, centroids)
    nc = get_program(pc)
    res = run_bass_kernel_spmd(nc, in_maps, list(range(NCORES)), **spmd_kwargs)
    out = np.array(np.asarray(centroids, dtype=np.float32), copy=True)
    for k, (ca, cb) in enumerate(meta):
        out[ca:cb] = res.results[k]["out_local"][: cb - ca]
    return out, res


def kernel(x, y, y_mask, centroids):
    out, _ = run(x, y, y_mask, centroids)
    return out
